# revision 14
# baseline (speedup 1.0000x reference)
"""Multi-head self-attention (B=16,T=512,C=1024,H=16) on 8 NeuronCores.

Strategy: data-parallel over batch (2 batches/core), no collectives.
All matmuls run in float32r (full PE rate at moving-dim >= 256).
Layout is chosen so no on-device transposes are needed:
  - QK projection emits [f, tok] (q^T / k^T per head are direct slices)
  - V projection swaps matmul operands to emit v as [tok, f]
  - scores are computed transposed: sT[kt, qt]; softmax sums arrive via a
    ones-column appended to v in the AV matmul; masking is a 0/1 multiply
    after exp (exact, since exp>0 and rows are never fully masked).
  - normalization (1/l) is broadcast across partitions via a DRAM bounce
    and folded into the PSUM->SBUF copy of the attention output.
"""

import math

import numpy as np

import concourse.bass as bass
import concourse.mybir as mybir
import concourse.tile as tile
from concourse import bacc
from concourse.bass_utils import run_bass_kernel_spmd

N_CORES = 8
B, T, C = 16, 512, 1024
H = 16
DH = C // H  # 64
B_LOC = B // N_CORES  # 2
TOK = B_LOC * T  # 1024 tokens per core
P = 128
CT = C // P  # 8 contraction tiles
FQK = 2 * C  # q+k rows
DT = mybir.dt.float16
F32 = mybir.dt.float32


def _build_nc():
    nc = bacc.Bacc("TRN2", target_bir_lowering=False, debug=False,
                   num_devices=N_CORES)

    xT = nc.dram_tensor("xT", [C, TOK], DT, kind="ExternalInput").ap()
    wqkT = nc.dram_tensor("wqkT", [C, FQK], DT, kind="ExternalInput").ap()
    wvT = nc.dram_tensor("wvT", [C, C], DT, kind="ExternalInput").ap()
    woT = nc.dram_tensor("woT", [C, C], DT, kind="ExternalInput").ap()
    maskT = nc.dram_tensor("maskT", [B_LOC, T, T], DT, kind="ExternalInput").ap()
    bias = nc.dram_tensor("bias", [C], F32, kind="ExternalInput").ap()
    out = nc.dram_tensor("out", [TOK, C], F32, kind="ExternalOutput").ap()

    with tile.TileContext(nc) as tc:
        _emit(nc, tc, xT, wqkT, wvT, woT, maskT, bias, out)

    nc.compile()
    return nc


def _emit(nc, tc, xT, wqkT, wvT, woT, maskT, bias, out):
    import os
    from contextlib import ExitStack
    PHASES = os.environ.get("KERNEL_PHASES", "full")
    ctx = ExitStack()
    with ctx:
        singles = ctx.enter_context(tc.tile_pool(name="singles", bufs=1))
        ps_a = ctx.enter_context(tc.tile_pool(name="ps_a", bufs=2, space="PSUM"))
        ps_s = ctx.enter_context(tc.tile_pool(name="ps_s", bufs=1, space="PSUM"))
        ps_o = ctx.enter_context(tc.tile_pool(name="ps_o", bufs=2, space="PSUM"))
        dram_pool = ctx.enter_context(
            tc.tile_pool(name="dscratch", bufs=2, space="DRAM"))

        # --- persistent SBUF tensors ---
        qk_sb = singles.tile([P, 16, TOK], DT)        # 64 KB/part
        v_sb = singles.tile([P, TOK // P, H, DH + 1], DT)  # 33.3 KB/part
        aoT_sb = singles.tile([P, CT, TOK], DT)       # 32 KB/part
        bias_sb = singles.tile([P, C], F32)           # 4 KB/part

        bias_bcast = bass.AP(tensor=bias.tensor, offset=bias.offset,
                             ap=[[0, P], *bias.ap])
        nc.gpsimd.dma_start(out=bias_sb[:], in_=bias_bcast)
        # ones column for the softmax row-sums
        nc.vector.memset(v_sb[:, :, :, DH:DH + 1], 1.0)

        with tc.tile_pool(name="xp", bufs=1) as x_pool, \
                tc.tile_pool(name="wq", bufs=3) as wq_pool, \
                tc.tile_pool(name="wv", bufs=1) as wv_pool:
            xT_sb = x_pool.tile([P, CT, TOK], DT)     # 32 KB/part
            nc.sync.dma_start(out=xT_sb[:],
                              in_=xT.rearrange("(k p) t -> p k t", p=P))

            # --- phase 1: QK projection -> qk_sb[f, tok] ---
            for j in range(16):
                wq = wq_pool.tile([P, CT, P], DT, tag="wq")
                nc.sync.dma_start(
                    out=wq[:],
                    in_=wqkT[:, j * P:(j + 1) * P].rearrange(
                        "(k p) f -> p k f", p=P))
                ps = [ps_a.tile([P, 512], F32, tag="ps_a", name=f"ps_qk_{j}_{tt}")
                      for tt in range(2)]
                for k in range(CT):
                    for tt in range(2):
                        nc.tensor.matmul(ps[tt][:], wq[:, k, :],
                                         xT_sb[:, k, tt * 512:(tt + 1) * 512],
                                         start=(k == 0), stop=(k == CT - 1))
                for tt in range(2):
                    nc.vector.tensor_copy(
                        out=qk_sb[:, j, tt * 512:(tt + 1) * 512],
                        in_=ps[tt][:])

            # --- phase 2: V projection -> v_sb[tok, h, d] (+ ones col) ---
            for n in range(2):  # fv tile of 512 = 8 heads
                wv = [wv_pool.tile([P, 512], DT, tag=f"wv_{k}", name=f"wv_{n}_{k}")
                      for k in range(CT)]
                for k in range(CT):
                    nc.sync.dma_start(
                        out=wv[k][:],
                        in_=wvT[k * P:(k + 1) * P, n * 512:(n + 1) * 512])
                for m in range(TOK // P):
                    ps = ps_a.tile([P, 512], F32, tag="ps_a", name=f"ps_v_{n}_{m}")
                    for k in range(CT):
                        nc.tensor.matmul(
                            ps[:], xT_sb[:, k, m * P:(m + 1) * P], wv[k][:],
                            start=(k == 0), stop=(k == CT - 1))
                    nc.vector.tensor_copy(
                        out=v_sb[:, m, 8 * n:8 * n + 8, 0:DH],
                        in_=ps[:].rearrange("p (h d) -> p h d", d=DH))

        if PHASES == "12":
            for m in range(TOK // P):
                nc.gpsimd.dma_start(
                    out=out[m * P:(m + 1) * P, :],
                    in_=v_sb[:, m, :, 0:DH])
            return

        with tc.tile_pool(name="mk", bufs=1) as mask_pool, \
                tc.tile_pool(name="pt", bufs=2) as pt_pool, \
                tc.tile_pool(name="linv", bufs=2) as linv_pool, \
                tc.tile_pool(name="aost", bufs=2) as ao_stage_pool:
            mask_sb = mask_pool.tile([P, B_LOC * (T // P), T], DT)
            nc.sync.dma_start(
                out=mask_sb[:],
                in_=maskT.rearrange("b (r p) q -> p (b r) q", p=P))

            # --- phase 3: attention per (b, h) ---
            NR = T // P  # 4 kt blocks
            for b in range(B_LOC):
                for h in range(H):
                    jq, jk, dlo = h // 2, 8 + h // 2, DH * (h % 2)
                    qT = qk_sb[dlo:dlo + DH, jq, b * T:(b + 1) * T]
                    sT = ps_s.tile([P, NR, 512], F32, tag="sT")
                    pT = pt_pool.tile([P, NR, 512], DT, tag="pT")
                    for r in range(NR):
                        kT = qk_sb[dlo:dlo + DH, jk,
                                   b * T + r * P: b * T + (r + 1) * P]
                        nc.tensor.matmul(sT[:, r, :], kT, qT,
                                         start=True, stop=True)
                        nc.scalar.activation(
                            out=pT[:, r, :], in_=sT[:, r, :],
                            func=mybir.ActivationFunctionType.Exp)
                        nc.vector.tensor_mul(out=pT[:, r, :], in0=pT[:, r, :],
                                             in1=mask_sb[:, b * NR + r, :])
                    po = ps_o.tile([P, 512], F32, tag="po")
                    for r in range(NR):
                        nc.tensor.matmul(po[0:DH + 1, :],
                                         v_sb[:, b * NR + r, h, :],
                                         pT[:, r, :],
                                         start=(r == 0), stop=(r == NR - 1))
                    # 1/l: recip in partition-major [128,4] (free-size-
                    # proportional cost), then partition-broadcast via DRAM
                    lrow = linv_pool.tile([P, 512], F32, tag="lrow")
                    nc.scalar.copy(out=lrow[DH:DH + 1, :],
                                   in_=po[DH:DH + 1, :])
                    lscr = dram_pool.tile([512], F32, tag="lscr")
                    nc.sync.dma_start(out=lscr[:], in_=lrow[DH:DH + 1, :])
                    lpart = linv_pool.tile([P, T // P], F32, tag="lpart")
                    nc.sync.dma_start(
                        out=lpart[:],
                        in_=lscr.rearrange("(qb p) -> p qb", p=P))
                    nc.vector.reciprocal(out=lpart[:, :], in_=lpart[:, :])
                    lscr2 = dram_pool.tile([512], F32, tag="lscr2")
                    nc.sync.dma_start(
                        out=lscr2.rearrange("(qb p) -> p qb", p=P),
                        in_=lpart[:])
                    l_bc = linv_pool.tile([DH, 512], F32, tag="l_bc")
                    lscr_bcast = bass.AP(tensor=lscr2.tensor, offset=lscr2.offset,
                                         ap=[[0, DH], *lscr2.ap])
                    nc.sync.dma_start(out=l_bc[:], in_=lscr_bcast)
                    if dlo == 0:
                        nc.vector.tensor_mul(
                            out=aoT_sb[0:DH, jq, b * T:(b + 1) * T],
                            in0=po[0:DH, :], in1=l_bc[:])
                    else:
                        ao_st = ao_stage_pool.tile([DH, 512], DT, tag="ao_st")
                        nc.vector.tensor_mul(out=ao_st[:], in0=po[0:DH, :],
                                             in1=l_bc[:])
                        nc.sync.dma_start(
                            out=aoT_sb[dlo:dlo + DH, jq, b * T:(b + 1) * T],
                            in_=ao_st[:])

        if PHASES == "123":
            for k in range(CT):
                nc.gpsimd.dma_start(
                    out=out[k * P:(k + 1) * P, :],
                    in_=aoT_sb[:, k, :])
            return

        with tc.tile_pool(name="wo", bufs=1) as wo_pool, \
                tc.tile_pool(name="y", bufs=3) as y_pool:
            # --- phase 4: out projection + bias ---
            for n in range(2):
                wo = [wo_pool.tile([P, 512], DT, tag=f"wo_{k}", name=f"wo_{n}_{k}")
                      for k in range(CT)]
                for k in range(CT):
                    nc.sync.dma_start(
                        out=wo[k][:],
                        in_=woT[k * P:(k + 1) * P, n * 512:(n + 1) * 512])
                for m in range(TOK // P):
                    ps = ps_a.tile([P, 512], F32, tag="ps_a", name=f"ps_y_{n}_{m}")
                    for k in range(CT):
                        nc.tensor.matmul(ps[:],
                                         aoT_sb[:, k, m * P:(m + 1) * P],
                                         wo[k][:],
                                         start=(k == 0), stop=(k == CT - 1))
                    y = y_pool.tile([P, 512], F32, tag="y")
                    nc.vector.tensor_add(out=y[:], in0=ps[:],
                                         in1=bias_sb[:, n * 512:(n + 1) * 512])
                    nc.sync.dma_start(
                        out=out[m * P:(m + 1) * P, n * 512:(n + 1) * 512],
                        in_=y[:])


_NC_CACHE = None


def _get_nc():
    global _NC_CACHE
    if _NC_CACHE is None:
        _NC_CACHE = _build_nc()
    return _NC_CACHE


def _prep_core_inputs(x, mask, key_padding_mask, w_qkv, w_out, b_out):
    """Host-side sharding + layout prep. Returns list of per-core in_maps."""
    x = np.asarray(x, dtype=np.float32)
    mask = np.asarray(mask)
    kpm = np.asarray(key_padding_mask)
    w_qkv = np.asarray(w_qkv, dtype=np.float32)
    w_out = np.asarray(w_out, dtype=np.float32)
    b_out = np.asarray(b_out, dtype=np.float32)

    scale = 1.0 / math.sqrt(DH)
    wqkT = w_qkv[:FQK].T.copy()  # [C, 2C]
    wqkT[:, :C] *= scale  # fold 1/sqrt(dh) into the Q weights
    wqkT = wqkT.astype(np.float16)
    wvT = np.ascontiguousarray(w_qkv[FQK:].T.astype(np.float16))  # [C, C]
    woT = np.ascontiguousarray(w_out.T.astype(np.float16))        # [C, C]
    maskTf = mask.T.astype(np.float16)         # [kt, qt]

    in_maps = []
    for i in range(N_CORES):
        xs = x[i * B_LOC:(i + 1) * B_LOC]      # [B_LOC, T, C]
        xT = np.ascontiguousarray(xs.reshape(TOK, C).T.astype(np.float16))
        kf = 1.0 - kpm[i * B_LOC:(i + 1) * B_LOC].astype(np.float16)  # [B_LOC,T]
        mT = (maskTf[None, :, :] * kf[:, :, None]).astype(np.float16)
        in_maps.append({
            "xT": xT,
            "wqkT": wqkT,
            "wvT": wvT,
            "woT": woT,
            "maskT": np.ascontiguousarray(mT),
            "bias": b_out,
        })
    return in_maps


def kernel(x, mask, key_padding_mask, w_qkv, w_out, b_out, _trace=False,
           _tmpdir=None):
    nc = _get_nc()
    in_maps = _prep_core_inputs(x, mask, key_padding_mask, w_qkv, w_out, b_out)
    res = run_bass_kernel_spmd(nc, in_maps, list(range(N_CORES)),
                               trace=_trace, tmpdir=_tmpdir)
    outs = [res.results[i]["out"].reshape(B_LOC, T, C) for i in range(N_CORES)]
    full = np.concatenate(outs, axis=0).astype(np.float32)
    kernel._last_exec_time_ns = res.exec_time_ns
    return full


# revision 16
# speedup vs baseline: 1.8212x; 1.8212x over previous
"""Multi-head self-attention (B=16,T=512,C=1024,H=16) on 8 NeuronCores.

Strategy: data-parallel over batch (2 batches/core), no collectives.
All matmuls run in float32r (full PE rate at moving-dim >= 256).
Layout is chosen so no on-device transposes are needed:
  - QK projection emits [f, tok] (q^T / k^T per head are direct slices)
  - V projection swaps matmul operands to emit v as [tok, f]
  - scores are computed transposed: sT[kt, qt]; softmax sums arrive via a
    ones-column appended to v in the AV matmul; masking is a 0/1 multiply
    after exp (exact, since exp>0 and rows are never fully masked).
  - normalization (1/l) is broadcast across partitions via a DRAM bounce
    and folded into the PSUM->SBUF copy of the attention output.
"""

import math

import numpy as np

import concourse.bass as bass
import concourse.mybir as mybir
import concourse.tile as tile
from concourse import bacc
from concourse.bass_utils import run_bass_kernel_spmd

N_CORES = 8
B, T, C = 16, 512, 1024
H = 16
DH = C // H  # 64
B_LOC = B // N_CORES  # 2
TOK = B_LOC * T  # 1024 tokens per core
P = 128
CT = C // P  # 8 contraction tiles
FQK = 2 * C  # q+k rows
DT = mybir.dt.float16
F32 = mybir.dt.float32


def _build_nc():
    nc = bacc.Bacc("TRN2", target_bir_lowering=False, debug=False,
                   num_devices=N_CORES)

    xT = nc.dram_tensor("xT", [C, TOK], DT, kind="ExternalInput").ap()
    wqkT = nc.dram_tensor("wqkT", [C, FQK], DT, kind="ExternalInput").ap()
    wvT = nc.dram_tensor("wvT", [C, C], DT, kind="ExternalInput").ap()
    woT = nc.dram_tensor("woT", [C, C], DT, kind="ExternalInput").ap()
    maskT = nc.dram_tensor("maskT", [B_LOC, T, T], DT, kind="ExternalInput").ap()
    bias = nc.dram_tensor("bias", [C], F32, kind="ExternalInput").ap()
    out = nc.dram_tensor("out", [TOK, C], F32, kind="ExternalOutput").ap()

    lall = nc.dram_tensor("lall", [2 * H, T], F32).ap()
    linv_scr = nc.dram_tensor("linv_scr", [2 * H, T], F32).ap()

    with tile.TileContext(nc) as tc:
        _emit(nc, tc, xT, wqkT, wvT, woT, maskT, bias, out, lall, linv_scr)

    nc.compile()
    return nc


def _emit(nc, tc, xT, wqkT, wvT, woT, maskT, bias, out, lall, linv_scr):
    import os
    from contextlib import ExitStack
    PHASES = os.environ.get("KERNEL_PHASES", "full")
    ctx = ExitStack()
    with ctx:
        singles = ctx.enter_context(tc.tile_pool(name="singles", bufs=1))
        ps_a = ctx.enter_context(tc.tile_pool(name="ps_a", bufs=2, space="PSUM"))
        ps_s = ctx.enter_context(tc.tile_pool(name="ps_s", bufs=1, space="PSUM"))
        ps_o = ctx.enter_context(tc.tile_pool(name="ps_o", bufs=2, space="PSUM"))
        dram_pool = ctx.enter_context(
            tc.tile_pool(name="dscratch", bufs=2, space="DRAM"))

        # --- persistent SBUF tensors ---
        qk_sb = singles.tile([P, 16, TOK], DT)        # 64 KB/part
        v_sb = singles.tile([P, TOK // P, H, DH + 1], DT)  # 33.3 KB/part
        aoT_sb = singles.tile([P, CT, TOK], DT)       # 32 KB/part
        bias_sb = singles.tile([P, C], F32)           # 4 KB/part

        bias_bcast = bass.AP(tensor=bias.tensor, offset=bias.offset,
                             ap=[[0, P], *bias.ap])
        nc.gpsimd.dma_start(out=bias_sb[:], in_=bias_bcast)
        # ones column for the softmax row-sums
        nc.vector.memset(v_sb[:, :, :, DH:DH + 1], 1.0)

        with tc.tile_pool(name="xp", bufs=1) as x_pool, \
                tc.tile_pool(name="wq", bufs=3) as wq_pool, \
                tc.tile_pool(name="wv", bufs=1) as wv_pool:
            xT_sb = x_pool.tile([P, CT, TOK], DT)     # 32 KB/part
            nc.sync.dma_start(out=xT_sb[:],
                              in_=xT.rearrange("(k p) t -> p k t", p=P))

            # --- phase 1: QK projection -> qk_sb[f, tok] ---
            for j in range(16):
                wq = wq_pool.tile([P, CT, P], DT, tag="wq")
                nc.sync.dma_start(
                    out=wq[:],
                    in_=wqkT[:, j * P:(j + 1) * P].rearrange(
                        "(k p) f -> p k f", p=P))
                ps = [ps_a.tile([P, 512], F32, tag="ps_a", name=f"ps_qk_{j}_{tt}")
                      for tt in range(2)]
                for k in range(CT):
                    for tt in range(2):
                        nc.tensor.matmul(ps[tt][:], wq[:, k, :],
                                         xT_sb[:, k, tt * 512:(tt + 1) * 512],
                                         start=(k == 0), stop=(k == CT - 1))
                for tt in range(2):
                    nc.vector.tensor_copy(
                        out=qk_sb[:, j, tt * 512:(tt + 1) * 512],
                        in_=ps[tt][:])

            # --- phase 2: V projection -> v_sb[tok, h, d] (+ ones col) ---
            for n in range(2):  # fv tile of 512 = 8 heads
                wv = [wv_pool.tile([P, 512], DT, tag=f"wv_{k}", name=f"wv_{n}_{k}")
                      for k in range(CT)]
                for k in range(CT):
                    nc.sync.dma_start(
                        out=wv[k][:],
                        in_=wvT[k * P:(k + 1) * P, n * 512:(n + 1) * 512])
                for m in range(TOK // P):
                    ps = ps_a.tile([P, 512], F32, tag="ps_a", name=f"ps_v_{n}_{m}")
                    for k in range(CT):
                        nc.tensor.matmul(
                            ps[:], xT_sb[:, k, m * P:(m + 1) * P], wv[k][:],
                            start=(k == 0), stop=(k == CT - 1))
                    nc.vector.tensor_copy(
                        out=v_sb[:, m, 8 * n:8 * n + 8, 0:DH],
                        in_=ps[:].rearrange("p (h d) -> p h d", d=DH))

        if PHASES == "12":
            for m in range(TOK // P):
                nc.gpsimd.dma_start(
                    out=out[m * P:(m + 1) * P, :],
                    in_=v_sb[:, m, :, 0:DH])
            return

        with tc.tile_pool(name="mk", bufs=1) as mask_pool, \
                tc.tile_pool(name="pt", bufs=2) as pt_pool, \
                tc.tile_pool(name="linv", bufs=2) as linv_pool, \
                tc.tile_pool(name="aost", bufs=2) as ao_stage_pool:
            mask_sb = mask_pool.tile([P, B_LOC * (T // P), T], DT)
            nc.sync.dma_start(
                out=mask_sb[:],
                in_=maskT.rearrange("b (r p) q -> p (b r) q", p=P))

            # --- phase 3: attention per (b, h) ---
            NR = T // P  # 4 kt blocks
            for b in range(B_LOC):
                for h in range(H):
                    jq, jk, dlo = h // 2, 8 + h // 2, DH * (h % 2)
                    qT = qk_sb[dlo:dlo + DH, jq, b * T:(b + 1) * T]
                    sT = ps_s.tile([P, NR, 512], F32, tag="sT")
                    pT = pt_pool.tile([P, NR, 512], DT, tag="pT")
                    for r in range(NR):
                        kT = qk_sb[dlo:dlo + DH, jk,
                                   b * T + r * P: b * T + (r + 1) * P]
                        nc.tensor.matmul(sT[:, r, :], kT, qT,
                                         start=True, stop=True)
                    nc.scalar.activation(
                        out=pT[:, :, :], in_=sT[:, :, :],
                        func=mybir.ActivationFunctionType.Exp)
                    nc.vector.tensor_mul(
                        out=pT[:, :, :], in0=pT[:, :, :],
                        in1=mask_sb[:, b * NR:(b + 1) * NR, :])
                    po = ps_o.tile([P, 512], F32, tag="po")
                    for r in range(NR):
                        nc.tensor.matmul(po[0:DH + 1, :],
                                         v_sb[:, b * NR + r, h, :],
                                         pT[:, r, :],
                                         start=(r == 0), stop=(r == NR - 1))
                    # stash row sums; normalization is batched after the loop
                    lrow = linv_pool.tile([P, 512], F32, tag="lrow")
                    nc.scalar.copy(out=lrow[DH:DH + 1, :],
                                   in_=po[DH:DH + 1, :])
                    nc.sync.dma_start(out=lall[2 * h + b, :],
                                      in_=lrow[DH:DH + 1, :])
                    if dlo == 0:
                        nc.vector.tensor_copy(
                            out=aoT_sb[0:DH, jq, b * T:(b + 1) * T],
                            in_=po[0:DH, :])
                    else:
                        ao_st = ao_stage_pool.tile([DH, 512], DT, tag="ao_st")
                        nc.vector.tensor_copy(out=ao_st[:], in_=po[0:DH, :])
                        nc.sync.dma_start(
                            out=aoT_sb[dlo:dlo + DH, jq, b * T:(b + 1) * T],
                            in_=ao_st[:])

            # batched softmax normalization: one reciprocal for all heads,
            # then 8 in-place multiplies over aoT c-tiles
            lpart = linv_pool.tile([P, 2 * H, T // P], F32, tag="lpart")
            nc.sync.dma_start(
                out=lpart[:],
                in_=lall.rearrange("h (qb p) -> p h qb", p=P))
            nc.vector.reciprocal(out=lpart[:], in_=lpart[:])
            nc.sync.dma_start(
                out=linv_scr.rearrange("h (qb p) -> p h qb", p=P),
                in_=lpart[:])
            for k in range(CT):
                lf = linv_pool.tile([P, TOK], F32, tag="lf")
                for half in range(2):
                    hh = 2 * k + half  # head index at partitions half*64
                    src_ap = bass.AP(
                        tensor=linv_scr.tensor,
                        offset=linv_scr.offset + 2 * hh * T,
                        ap=[[0, DH], [1, TOK]])
                    nc.sync.dma_start(out=lf[half * DH:(half + 1) * DH, :],
                                      in_=src_ap)
                nc.vector.tensor_mul(out=aoT_sb[:, k, :],
                                     in0=aoT_sb[:, k, :], in1=lf[:])

        if PHASES == "123":
            for k in range(CT):
                nc.gpsimd.dma_start(
                    out=out[k * P:(k + 1) * P, :],
                    in_=aoT_sb[:, k, :])
            return

        with tc.tile_pool(name="wo", bufs=1) as wo_pool, \
                tc.tile_pool(name="y", bufs=3) as y_pool:
            # --- phase 4: out projection + bias ---
            for n in range(2):
                wo = [wo_pool.tile([P, 512], DT, tag=f"wo_{k}", name=f"wo_{n}_{k}")
                      for k in range(CT)]
                for k in range(CT):
                    nc.sync.dma_start(
                        out=wo[k][:],
                        in_=woT[k * P:(k + 1) * P, n * 512:(n + 1) * 512])
                for m in range(TOK // P):
                    ps = ps_a.tile([P, 512], F32, tag="ps_a", name=f"ps_y_{n}_{m}")
                    for k in range(CT):
                        nc.tensor.matmul(ps[:],
                                         aoT_sb[:, k, m * P:(m + 1) * P],
                                         wo[k][:],
                                         start=(k == 0), stop=(k == CT - 1))
                    y = y_pool.tile([P, 512], F32, tag="y")
                    nc.vector.tensor_add(out=y[:], in0=ps[:],
                                         in1=bias_sb[:, n * 512:(n + 1) * 512])
                    nc.sync.dma_start(
                        out=out[m * P:(m + 1) * P, n * 512:(n + 1) * 512],
                        in_=y[:])


_NC_CACHE = None


def _get_nc():
    global _NC_CACHE
    if _NC_CACHE is None:
        _NC_CACHE = _build_nc()
    return _NC_CACHE


def _prep_core_inputs(x, mask, key_padding_mask, w_qkv, w_out, b_out):
    """Host-side sharding + layout prep. Returns list of per-core in_maps."""
    x = np.asarray(x, dtype=np.float32)
    mask = np.asarray(mask)
    kpm = np.asarray(key_padding_mask)
    w_qkv = np.asarray(w_qkv, dtype=np.float32)
    w_out = np.asarray(w_out, dtype=np.float32)
    b_out = np.asarray(b_out, dtype=np.float32)

    scale = 1.0 / math.sqrt(DH)
    wqkT = w_qkv[:FQK].T.copy()  # [C, 2C]
    wqkT[:, :C] *= scale  # fold 1/sqrt(dh) into the Q weights
    wqkT = wqkT.astype(np.float16)
    wvT = np.ascontiguousarray(w_qkv[FQK:].T.astype(np.float16))  # [C, C]
    woT = np.ascontiguousarray(w_out.T.astype(np.float16))        # [C, C]
    maskTf = mask.T.astype(np.float16)         # [kt, qt]

    in_maps = []
    for i in range(N_CORES):
        xs = x[i * B_LOC:(i + 1) * B_LOC]      # [B_LOC, T, C]
        xT = np.ascontiguousarray(xs.reshape(TOK, C).T.astype(np.float16))
        kf = 1.0 - kpm[i * B_LOC:(i + 1) * B_LOC].astype(np.float16)  # [B_LOC,T]
        mT = (maskTf[None, :, :] * kf[:, :, None]).astype(np.float16)
        in_maps.append({
            "xT": xT,
            "wqkT": wqkT,
            "wvT": wvT,
            "woT": woT,
            "maskT": np.ascontiguousarray(mT),
            "bias": b_out,
        })
    return in_maps


def kernel(x, mask, key_padding_mask, w_qkv, w_out, b_out, _trace=False,
           _tmpdir=None):
    nc = _get_nc()
    in_maps = _prep_core_inputs(x, mask, key_padding_mask, w_qkv, w_out, b_out)
    res = run_bass_kernel_spmd(nc, in_maps, list(range(N_CORES)),
                               trace=_trace, tmpdir=_tmpdir)
    outs = [res.results[i]["out"].reshape(B_LOC, T, C) for i in range(N_CORES)]
    full = np.concatenate(outs, axis=0).astype(np.float32)
    kernel._last_exec_time_ns = res.exec_time_ns
    return full


# revision 17
# speedup vs baseline: 1.9468x; 1.0690x over previous
"""Multi-head self-attention (B=16,T=512,C=1024,H=16) on 8 NeuronCores.

Strategy: data-parallel over batch (2 batches/core), no collectives.
All matmuls run in float32r (full PE rate at moving-dim >= 256).
Layout is chosen so no on-device transposes are needed:
  - QK projection emits [f, tok] (q^T / k^T per head are direct slices)
  - V projection swaps matmul operands to emit v as [tok, f]
  - scores are computed transposed: sT[kt, qt]; softmax sums arrive via a
    ones-column appended to v in the AV matmul; masking is a 0/1 multiply
    after exp (exact, since exp>0 and rows are never fully masked).
  - normalization (1/l) is broadcast across partitions via a DRAM bounce
    and folded into the PSUM->SBUF copy of the attention output.
"""

import math

import numpy as np

import concourse.bass as bass
import concourse.mybir as mybir
import concourse.tile as tile
from concourse import bacc
from concourse.bass_utils import run_bass_kernel_spmd

N_CORES = 8
B, T, C = 16, 512, 1024
H = 16
DH = C // H  # 64
B_LOC = B // N_CORES  # 2
TOK = B_LOC * T  # 1024 tokens per core
P = 128
CT = C // P  # 8 contraction tiles
FQK = 2 * C  # q+k rows
DT = mybir.dt.float16
F32 = mybir.dt.float32


def _build_nc():
    nc = bacc.Bacc("TRN2", target_bir_lowering=False, debug=False,
                   num_devices=N_CORES)

    xT = nc.dram_tensor("xT", [C, TOK], DT, kind="ExternalInput").ap()
    wqkT = nc.dram_tensor("wqkT", [C, FQK], DT, kind="ExternalInput").ap()
    wvT = nc.dram_tensor("wvT", [C, C], DT, kind="ExternalInput").ap()
    woT = nc.dram_tensor("woT", [C, C], DT, kind="ExternalInput").ap()
    maskT = nc.dram_tensor("maskT", [B_LOC, T, T], DT, kind="ExternalInput").ap()
    bias = nc.dram_tensor("bias", [C], F32, kind="ExternalInput").ap()
    out = nc.dram_tensor("out", [TOK, C], F32, kind="ExternalOutput").ap()

    lall = nc.dram_tensor("lall", [B_LOC, H, T], F32).ap()
    linv_scr = nc.dram_tensor("linv_scr", [B_LOC, H, T], F32).ap()

    with tile.TileContext(nc) as tc:
        _emit(nc, tc, xT, wqkT, wvT, woT, maskT, bias, out, lall, linv_scr)

    nc.compile()
    return nc


def _emit(nc, tc, xT, wqkT, wvT, woT, maskT, bias, out, lall, linv_scr):
    from contextlib import ExitStack
    ctx = ExitStack()
    with ctx:
        singles = ctx.enter_context(tc.tile_pool(name="singles", bufs=1))
        wo_pool = ctx.enter_context(tc.tile_pool(name="wo", bufs=1))
        ps_a = ctx.enter_context(tc.tile_pool(name="ps_a", bufs=2, space="PSUM"))
        ps_s = ctx.enter_context(tc.tile_pool(name="ps_s", bufs=4, space="PSUM"))
        ps_o = ctx.enter_context(tc.tile_pool(name="ps_o", bufs=2, space="PSUM"))
        pt_pool = ctx.enter_context(tc.tile_pool(name="pt", bufs=2))
        linv_pool = ctx.enter_context(tc.tile_pool(name="linv", bufs=2))
        ao_stage_pool = ctx.enter_context(tc.tile_pool(name="aost", bufs=2))
        y_pool = ctx.enter_context(tc.tile_pool(name="y", bufs=3))

        # --- persistent SBUF tensors ---
        qk_sb = singles.tile([P, 16, TOK], DT)             # 32 KB/part
        v_sb = singles.tile([P, TOK // P, H, DH + 1], DT)  # 16.6 KB/part
        ao_b = [singles.tile([P, CT, T], DT, name=f"ao_b{b}")
                for b in range(B_LOC)]                     # 2x 8 KB/part
        bias_sb = singles.tile([P, C], F32)                # 4 KB/part
        mask_sb = singles.tile([P, B_LOC * (T // P), T], DT)  # 8 KB/part

        bias_bcast = bass.AP(tensor=bias.tensor, offset=bias.offset,
                             ap=[[0, P], *bias.ap])
        nc.gpsimd.dma_start(out=bias_sb[:], in_=bias_bcast)
        nc.sync.dma_start(
            out=mask_sb[:],
            in_=maskT.rearrange("b (r p) q -> p (b r) q", p=P))
        # ones column for the softmax row-sums
        nc.vector.memset(v_sb[:, :, :, DH:DH + 1], 1.0)
        # prefetch out-projection weights early (no deps)
        wo = [wo_pool.tile([P, 512], DT, tag=f"wo_{n}_{k}", name=f"wo_{n}_{k}")
              for n in range(2) for k in range(CT)]
        for n in range(2):
            for k in range(CT):
                nc.sync.dma_start(
                    out=wo[n * CT + k][:],
                    in_=woT[k * P:(k + 1) * P, n * 512:(n + 1) * 512])

        with tc.tile_pool(name="xp", bufs=1) as x_pool, \
                tc.tile_pool(name="wq", bufs=3) as wq_pool, \
                tc.tile_pool(name="wv", bufs=1) as wv_pool:
            xT_sb = x_pool.tile([P, CT, TOK], DT)          # 16 KB/part
            nc.sync.dma_start(out=xT_sb[:],
                              in_=xT.rearrange("(k p) t -> p k t", p=P))

            # --- phase 1: QK projection -> qk_sb[f, tok] ---
            for j in range(16):
                wq = wq_pool.tile([P, CT, P], DT, tag="wq")
                nc.sync.dma_start(
                    out=wq[:],
                    in_=wqkT[:, j * P:(j + 1) * P].rearrange(
                        "(k p) f -> p k f", p=P))
                ps = [ps_a.tile([P, 512], F32, tag="ps_a", name=f"ps_qk_{j}_{tt}")
                      for tt in range(2)]
                for k in range(CT):
                    for tt in range(2):
                        nc.tensor.matmul(ps[tt][:], wq[:, k, :],
                                         xT_sb[:, k, tt * 512:(tt + 1) * 512],
                                         start=(k == 0), stop=(k == CT - 1))
                for tt in range(2):
                    nc.vector.tensor_copy(
                        out=qk_sb[:, j, tt * 512:(tt + 1) * 512],
                        in_=ps[tt][:])

            # --- phase 2: V projection -> v_sb[tok, h, d] (+ ones col) ---
            for n in range(2):
                wv = [wv_pool.tile([P, 512], DT, tag=f"wv_{k}",
                                   name=f"wv_{n}_{k}") for k in range(CT)]
                for k in range(CT):
                    nc.sync.dma_start(
                        out=wv[k][:],
                        in_=wvT[k * P:(k + 1) * P, n * 512:(n + 1) * 512])
                for m in range(TOK // P):
                    ps = ps_a.tile([P, 512], F32, tag="ps_a", name=f"ps_v_{n}_{m}")
                    for k in range(CT):
                        nc.tensor.matmul(
                            ps[:], xT_sb[:, k, m * P:(m + 1) * P], wv[k][:],
                            start=(k == 0), stop=(k == CT - 1))
                    nc.vector.tensor_copy(
                        out=v_sb[:, m, 8 * n:8 * n + 8, 0:DH],
                        in_=ps[:].rearrange("p (h d) -> p h d", d=DH))

        # --- phase 3+4 interleaved per batch ---
        NR = T // P  # 4 kt blocks
        for b in range(B_LOC):
            for h in range(H):
                jq, jk, dlo = h // 2, 8 + h // 2, DH * (h % 2)
                qT = qk_sb[dlo:dlo + DH, jq, b * T:(b + 1) * T]
                pT = pt_pool.tile([P, NR, 512], DT, tag="pT")
                sT = [ps_s.tile([P, 512], F32, tag="sT", name=f"sT_{b}_{h}_{r}")
                      for r in range(NR)]
                for r in range(NR):
                    kT = qk_sb[dlo:dlo + DH, jk,
                               b * T + r * P: b * T + (r + 1) * P]
                    nc.tensor.matmul(sT[r][:], kT, qT, start=True, stop=True)
                    nc.scalar.activation(
                        out=pT[:, r, :], in_=sT[r][:],
                        func=mybir.ActivationFunctionType.Exp)
                    nc.vector.tensor_mul(out=pT[:, r, :], in0=pT[:, r, :],
                                         in1=mask_sb[:, b * NR + r, :])
                po = ps_o.tile([P, 512], F32, tag="po")
                for r in range(NR):
                    nc.tensor.matmul(po[0:DH + 1, :],
                                     v_sb[:, b * NR + r, h, :], pT[:, r, :],
                                     start=(r == 0), stop=(r == NR - 1))
                # stash row sums; normalization is batched per b
                lrow = linv_pool.tile([P, 512], F32, tag="lrow")
                nc.vector.tensor_copy(out=lrow[DH:DH + 1, :],
                                      in_=po[DH:DH + 1, :])
                nc.sync.dma_start(out=lall[b, h, :], in_=lrow[DH:DH + 1, :])
                if dlo == 0:
                    nc.vector.tensor_copy(
                        out=ao_b[b][0:DH, jq, :], in_=po[0:DH, :])
                else:
                    ao_st = ao_stage_pool.tile([DH, 512], DT, tag="ao_st")
                    nc.vector.tensor_copy(out=ao_st[:], in_=po[0:DH, :])
                    nc.sync.dma_start(out=ao_b[b][dlo:dlo + DH, jq, :],
                                      in_=ao_st[:])

            # batched 1/l for this b, then in-place normalize ao_b[b]
            lpart = linv_pool.tile([P, H, T // P], F32, tag="lpart")
            nc.sync.dma_start(
                out=lpart[:],
                in_=lall[b].rearrange("h (qb p) -> p h qb", p=P))
            nc.vector.reciprocal(out=lpart[:], in_=lpart[:])
            nc.sync.dma_start(
                out=linv_scr[b].rearrange("h (qb p) -> p h qb", p=P),
                in_=lpart[:])
            for k in range(CT):
                lf = linv_pool.tile([P, T], F32, tag="lf")
                for half in range(2):
                    hh = 2 * k + half
                    src_ap = bass.AP(
                        tensor=linv_scr.tensor,
                        offset=linv_scr.offset + (b * H + hh) * T,
                        ap=[[0, DH], [1, T]])
                    nc.sync.dma_start(out=lf[half * DH:(half + 1) * DH, :],
                                      in_=src_ap)
                nc.vector.tensor_mul(out=ao_b[b][:, k, :],
                                     in0=ao_b[b][:, k, :], in1=lf[:])

            # --- out projection + bias for this b ---
            for n in range(2):
                for m in range(T // P):
                    ps = ps_a.tile([P, 512], F32, tag="ps_a",
                                   name=f"ps_y_{b}_{n}_{m}")
                    for k in range(CT):
                        nc.tensor.matmul(ps[:], ao_b[b][:, k, m * P:(m + 1) * P],
                                         wo[n * CT + k][:],
                                         start=(k == 0), stop=(k == CT - 1))
                    y = y_pool.tile([P, 512], F32, tag="y")
                    nc.vector.tensor_add(out=y[:], in0=ps[:],
                                         in1=bias_sb[:, n * 512:(n + 1) * 512])
                    nc.sync.dma_start(
                        out=out[b * T + m * P: b * T + (m + 1) * P,
                                n * 512:(n + 1) * 512],
                        in_=y[:])


_NC_CACHE = None


def _get_nc():
    global _NC_CACHE
    if _NC_CACHE is None:
        _NC_CACHE = _build_nc()
    return _NC_CACHE


def _prep_core_inputs(x, mask, key_padding_mask, w_qkv, w_out, b_out):
    """Host-side sharding + layout prep. Returns list of per-core in_maps."""
    x = np.asarray(x, dtype=np.float32)
    mask = np.asarray(mask)
    kpm = np.asarray(key_padding_mask)
    w_qkv = np.asarray(w_qkv, dtype=np.float32)
    w_out = np.asarray(w_out, dtype=np.float32)
    b_out = np.asarray(b_out, dtype=np.float32)

    scale = 1.0 / math.sqrt(DH)
    wqkT = w_qkv[:FQK].T.copy()  # [C, 2C]
    wqkT[:, :C] *= scale  # fold 1/sqrt(dh) into the Q weights
    wqkT = wqkT.astype(np.float16)
    wvT = np.ascontiguousarray(w_qkv[FQK:].T.astype(np.float16))  # [C, C]
    woT = np.ascontiguousarray(w_out.T.astype(np.float16))        # [C, C]
    maskTf = mask.T.astype(np.float16)         # [kt, qt]

    in_maps = []
    for i in range(N_CORES):
        xs = x[i * B_LOC:(i + 1) * B_LOC]      # [B_LOC, T, C]
        xT = np.ascontiguousarray(xs.reshape(TOK, C).T.astype(np.float16))
        kf = 1.0 - kpm[i * B_LOC:(i + 1) * B_LOC].astype(np.float16)  # [B_LOC,T]
        mT = (maskTf[None, :, :] * kf[:, :, None]).astype(np.float16)
        in_maps.append({
            "xT": xT,
            "wqkT": wqkT,
            "wvT": wvT,
            "woT": woT,
            "maskT": np.ascontiguousarray(mT),
            "bias": b_out,
        })
    return in_maps


def kernel(x, mask, key_padding_mask, w_qkv, w_out, b_out, _trace=False,
           _tmpdir=None):
    nc = _get_nc()
    in_maps = _prep_core_inputs(x, mask, key_padding_mask, w_qkv, w_out, b_out)
    res = run_bass_kernel_spmd(nc, in_maps, list(range(N_CORES)),
                               trace=_trace, tmpdir=_tmpdir)
    outs = [res.results[i]["out"].reshape(B_LOC, T, C) for i in range(N_CORES)]
    full = np.concatenate(outs, axis=0).astype(np.float32)
    kernel._last_exec_time_ns = res.exec_time_ns
    return full


# revision 18
# speedup vs baseline: 2.4560x; 1.2616x over previous
"""Multi-head self-attention (B=16,T=512,C=1024,H=16) on 8 NeuronCores.

Strategy: data-parallel over batch (2 batches/core), no collectives.
All matmuls run in float32r (full PE rate at moving-dim >= 256).
Layout is chosen so no on-device transposes are needed:
  - QK projection emits [f, tok] (q^T / k^T per head are direct slices)
  - V projection swaps matmul operands to emit v as [tok, f]
  - scores are computed transposed: sT[kt, qt]; softmax sums arrive via a
    ones-column appended to v in the AV matmul; masking is a 0/1 multiply
    after exp (exact, since exp>0 and rows are never fully masked).
  - normalization (1/l) is broadcast across partitions via a DRAM bounce
    and folded into the PSUM->SBUF copy of the attention output.
"""

import math

import numpy as np

import concourse.bass as bass
import concourse.mybir as mybir
import concourse.tile as tile
from concourse import bacc
from concourse.bass_utils import run_bass_kernel_spmd

N_CORES = 8
B, T, C = 16, 512, 1024
H = 16
DH = C // H  # 64
B_LOC = B // N_CORES  # 2
TOK = B_LOC * T  # 1024 tokens per core
P = 128
CT = C // P  # 8 contraction tiles
FQK = 2 * C  # q+k rows
DT = mybir.dt.float16
F32 = mybir.dt.float32


def _build_nc():
    nc = bacc.Bacc("TRN2", target_bir_lowering=False, debug=False,
                   num_devices=N_CORES)

    xT = nc.dram_tensor("xT", [C, TOK], DT, kind="ExternalInput").ap()
    wqkT = nc.dram_tensor("wqkT", [C, FQK], DT, kind="ExternalInput").ap()
    wvT = nc.dram_tensor("wvT", [C, C], DT, kind="ExternalInput").ap()
    woT = nc.dram_tensor("woT", [C, C], DT, kind="ExternalInput").ap()
    maskT = nc.dram_tensor("maskT", [B_LOC, T, T], DT, kind="ExternalInput").ap()
    bias = nc.dram_tensor("bias", [C], F32, kind="ExternalInput").ap()
    out = nc.dram_tensor("out", [TOK, C], F32, kind="ExternalOutput").ap()

    lall = nc.dram_tensor("lall", [B_LOC, H, T], F32).ap()
    linv_scr = nc.dram_tensor("linv_scr", [B_LOC, H, T], F32).ap()

    with tile.TileContext(nc) as tc:
        _emit(nc, tc, xT, wqkT, wvT, woT, maskT, bias, out, lall, linv_scr)

    nc.compile()
    return nc


def _emit(nc, tc, xT, wqkT, wvT, woT, maskT, bias, out, lall, linv_scr):
    from contextlib import ExitStack
    ctx = ExitStack()
    with ctx:
        singles = ctx.enter_context(tc.tile_pool(name="singles", bufs=1))
        wo_pool = ctx.enter_context(tc.tile_pool(name="wo", bufs=1))
        ps_a = ctx.enter_context(tc.tile_pool(name="ps_a", bufs=2, space="PSUM"))
        ps_s = ctx.enter_context(tc.tile_pool(name="ps_s", bufs=4, space="PSUM"))
        ps_o = ctx.enter_context(tc.tile_pool(name="ps_o", bufs=2, space="PSUM"))
        pt_pool = ctx.enter_context(tc.tile_pool(name="pt", bufs=2))
        linv_pool = ctx.enter_context(tc.tile_pool(name="linv", bufs=2))
        ao_stage_pool = ctx.enter_context(tc.tile_pool(name="aost", bufs=2))
        y_pool = ctx.enter_context(tc.tile_pool(name="y", bufs=3))

        # --- persistent SBUF tensors ---
        qk_sb = singles.tile([P, 16, TOK], DT)             # 32 KB/part
        v_sb = singles.tile([P, TOK // P, H, DH + 1], DT)  # 16.6 KB/part
        ao_b = [singles.tile([P, CT, T], DT, name=f"ao_b{b}")
                for b in range(B_LOC)]                     # 2x 8 KB/part
        bias_sb = singles.tile([P, C], F32)                # 4 KB/part
        mask_sb = singles.tile([P, B_LOC * (T // P), T], DT)  # 8 KB/part

        bias_bcast = bass.AP(tensor=bias.tensor, offset=bias.offset,
                             ap=[[0, P], *bias.ap])
        nc.gpsimd.dma_start(out=bias_sb[:], in_=bias_bcast)
        nc.sync.dma_start(
            out=mask_sb[:],
            in_=maskT.rearrange("b (r p) q -> p (b r) q", p=P))
        # ones column for the softmax row-sums
        nc.vector.memset(v_sb[:, :, :, DH:DH + 1], 1.0)
        # prefetch out-projection weights early (no deps)
        wo = [wo_pool.tile([P, 512], DT, tag=f"wo_{n}_{k}", name=f"wo_{n}_{k}")
              for n in range(2) for k in range(CT)]
        for n in range(2):
            for k in range(CT):
                nc.sync.dma_start(
                    out=wo[n * CT + k][:],
                    in_=woT[k * P:(k + 1) * P, n * 512:(n + 1) * 512])

        with tc.tile_pool(name="xp", bufs=1) as x_pool, \
                tc.tile_pool(name="wq", bufs=3) as wq_pool, \
                tc.tile_pool(name="wv", bufs=1) as wv_pool:
            xT_sb = x_pool.tile([P, CT, TOK], DT)          # 16 KB/part
            nc.sync.dma_start(out=xT_sb[:],
                              in_=xT.rearrange("(k p) t -> p k t", p=P))

            # --- phase 1: QK projection -> qk_sb[f, tok] ---
            for j in range(16):
                wq = wq_pool.tile([P, CT, P], DT, tag="wq")
                nc.sync.dma_start(
                    out=wq[:],
                    in_=wqkT[:, j * P:(j + 1) * P].rearrange(
                        "(k p) f -> p k f", p=P))
                ps = [ps_a.tile([P, 512], F32, tag="ps_a", name=f"ps_qk_{j}_{tt}")
                      for tt in range(2)]
                for k in range(CT):
                    for tt in range(2):
                        nc.tensor.matmul(ps[tt][:], wq[:, k, :],
                                         xT_sb[:, k, tt * 512:(tt + 1) * 512],
                                         start=(k == 0), stop=(k == CT - 1))
                for tt in range(2):
                    nc.vector.tensor_copy(
                        out=qk_sb[:, j, tt * 512:(tt + 1) * 512],
                        in_=ps[tt][:])

            # --- phase 2: V projection -> v_sb[tok, h, d] (+ ones col) ---
            for n in range(2):
                wv = [wv_pool.tile([P, 512], DT, tag=f"wv_{k}",
                                   name=f"wv_{n}_{k}") for k in range(CT)]
                for k in range(CT):
                    nc.sync.dma_start(
                        out=wv[k][:],
                        in_=wvT[k * P:(k + 1) * P, n * 512:(n + 1) * 512])
                for m in range(TOK // P):
                    ps = ps_a.tile([P, 512], F32, tag="ps_a", name=f"ps_v_{n}_{m}")
                    for k in range(CT):
                        nc.tensor.matmul(
                            ps[:], xT_sb[:, k, m * P:(m + 1) * P], wv[k][:],
                            start=(k == 0), stop=(k == CT - 1))
                    nc.vector.tensor_copy(
                        out=v_sb[:, m, 8 * n:8 * n + 8, 0:DH],
                        in_=ps[:].rearrange("p (h d) -> p h d", d=DH))

        # --- phase 3+4 interleaved per batch ---
        NR = T // P  # 4 kt blocks
        for b in range(B_LOC):
            for h in range(H):
                jq, jk, dlo = h // 2, 8 + h // 2, DH * (h % 2)
                qT = qk_sb[dlo:dlo + DH, jq, b * T:(b + 1) * T]
                pT = pt_pool.tile([P, NR, 512], DT, tag="pT")
                sT = [ps_s.tile([P, 512], F32, tag="sT", name=f"sT_{b}_{h}_{r}")
                      for r in range(NR)]
                for r in range(NR):
                    kT = qk_sb[dlo:dlo + DH, jk,
                               b * T + r * P: b * T + (r + 1) * P]
                    nc.tensor.matmul(sT[r][:], kT, qT, start=True, stop=True)
                    nc.scalar.activation(
                        out=pT[:, r, :], in_=sT[r][:],
                        func=mybir.ActivationFunctionType.Exp)
                    nc.vector.tensor_mul(out=pT[:, r, :], in0=pT[:, r, :],
                                         in1=mask_sb[:, b * NR + r, :])
                po = ps_o.tile([P, 512], F32, tag="po")
                for r in range(NR):
                    nc.tensor.matmul(po[0:DH + 1, :],
                                     v_sb[:, b * NR + r, h, :], pT[:, r, :],
                                     start=(r == 0), stop=(r == NR - 1))
                # stash row sums; normalization is batched per b
                lrow = linv_pool.tile([P, 512], F32, tag="lrow")
                nc.vector.tensor_copy(out=lrow[DH:DH + 1, :],
                                      in_=po[DH:DH + 1, :])
                nc.sync.dma_start(out=lall[b, h, :], in_=lrow[DH:DH + 1, :])
                if dlo == 0:
                    nc.vector.tensor_copy(
                        out=ao_b[b][0:DH, jq, :], in_=po[0:DH, :])
                else:
                    ao_st = ao_stage_pool.tile([DH, 512], DT, tag="ao_st")
                    nc.vector.tensor_copy(out=ao_st[:], in_=po[0:DH, :])
                    nc.sync.dma_start(out=ao_b[b][dlo:dlo + DH, jq, :],
                                      in_=ao_st[:])

            # batched 1/l for this b, then in-place normalize ao_b[b]
            lpart = linv_pool.tile([H, T], F32, tag="lpart")
            nc.sync.dma_start(out=lpart[:], in_=lall[b])
            nc.vector.reciprocal(out=lpart[:], in_=lpart[:])
            nc.sync.dma_start(out=linv_scr[b], in_=lpart[:])
            for k in range(CT):
                lf = linv_pool.tile([P, T], F32, tag="lf")
                for half in range(2):
                    hh = 2 * k + half
                    src_ap = bass.AP(
                        tensor=linv_scr.tensor,
                        offset=linv_scr.offset + (b * H + hh) * T,
                        ap=[[0, DH], [1, T]])
                    nc.sync.dma_start(out=lf[half * DH:(half + 1) * DH, :],
                                      in_=src_ap)
                nc.vector.tensor_mul(out=ao_b[b][:, k, :],
                                     in0=ao_b[b][:, k, :], in1=lf[:])

            # --- out projection + bias for this b ---
            for n in range(2):
                for m in range(T // P):
                    ps = ps_a.tile([P, 512], F32, tag="ps_a",
                                   name=f"ps_y_{b}_{n}_{m}")
                    for k in range(CT):
                        nc.tensor.matmul(ps[:], ao_b[b][:, k, m * P:(m + 1) * P],
                                         wo[n * CT + k][:],
                                         start=(k == 0), stop=(k == CT - 1))
                    y = y_pool.tile([P, 512], F32, tag="y")
                    nc.vector.tensor_add(out=y[:], in0=ps[:],
                                         in1=bias_sb[:, n * 512:(n + 1) * 512])
                    nc.sync.dma_start(
                        out=out[b * T + m * P: b * T + (m + 1) * P,
                                n * 512:(n + 1) * 512],
                        in_=y[:])


_NC_CACHE = None


def _get_nc():
    global _NC_CACHE
    if _NC_CACHE is None:
        _NC_CACHE = _build_nc()
    return _NC_CACHE


def _prep_core_inputs(x, mask, key_padding_mask, w_qkv, w_out, b_out):
    """Host-side sharding + layout prep. Returns list of per-core in_maps."""
    x = np.asarray(x, dtype=np.float32)
    mask = np.asarray(mask)
    kpm = np.asarray(key_padding_mask)
    w_qkv = np.asarray(w_qkv, dtype=np.float32)
    w_out = np.asarray(w_out, dtype=np.float32)
    b_out = np.asarray(b_out, dtype=np.float32)

    scale = 1.0 / math.sqrt(DH)
    wqkT = w_qkv[:FQK].T.copy()  # [C, 2C]
    wqkT[:, :C] *= scale  # fold 1/sqrt(dh) into the Q weights
    wqkT = wqkT.astype(np.float16)
    wvT = np.ascontiguousarray(w_qkv[FQK:].T.astype(np.float16))  # [C, C]
    woT = np.ascontiguousarray(w_out.T.astype(np.float16))        # [C, C]
    maskTf = mask.T.astype(np.float16)         # [kt, qt]

    in_maps = []
    for i in range(N_CORES):
        xs = x[i * B_LOC:(i + 1) * B_LOC]      # [B_LOC, T, C]
        xT = np.ascontiguousarray(xs.reshape(TOK, C).T.astype(np.float16))
        kf = 1.0 - kpm[i * B_LOC:(i + 1) * B_LOC].astype(np.float16)  # [B_LOC,T]
        mT = (maskTf[None, :, :] * kf[:, :, None]).astype(np.float16)
        in_maps.append({
            "xT": xT,
            "wqkT": wqkT,
            "wvT": wvT,
            "woT": woT,
            "maskT": np.ascontiguousarray(mT),
            "bias": b_out,
        })
    return in_maps


def kernel(x, mask, key_padding_mask, w_qkv, w_out, b_out, _trace=False,
           _tmpdir=None):
    nc = _get_nc()
    in_maps = _prep_core_inputs(x, mask, key_padding_mask, w_qkv, w_out, b_out)
    res = run_bass_kernel_spmd(nc, in_maps, list(range(N_CORES)),
                               trace=_trace, tmpdir=_tmpdir)
    outs = [res.results[i]["out"].reshape(B_LOC, T, C) for i in range(N_CORES)]
    full = np.concatenate(outs, axis=0).astype(np.float32)
    kernel._last_exec_time_ns = res.exec_time_ns
    return full
